# revision 43
# baseline (speedup 1.0000x reference)
"""Trainium2 Bass kernel for nn_LogicLayer (ProductTNorm 'and' LogicLayer forward).

Math: y[b,o] = prod_i (1 - v[o,i]*u[b,i]),  v = sigmoid(w), u = 1 - atoms.
ln y[b,o] = sum_i ln(1 - v*u) ~= I*c0 + sum_{k=1..K} c_k * sum_i v^k[o,i] u^k[b,i]
so each polynomial term is a (B,I)x(I,O) matmul and the whole reduction runs on
TensorE instead of elementwise Ln on ScalarE (the 265us baseline approach).

Coefficients c_k: weighted least-squares fit of ln(1-x) on the input
distribution (weight ~ y^2 = the norm-relative metric), fitted against the
fp16-quantized basis the device actually computes (see fit_coeffs.py).

Per-core layout (8 cores, data-parallel over batch, B_loc=512):
  * inputs: a16T = fp16(atoms.T) slice (I, B_loc), lnvT = fp16(softplus(-w).T)
    (I, O); input DMA triggers split across the sync and scalar HWDGE queues.
  * moving side (DVE): plain fp16 tensor_mul chain m_k = m_{k-1} * base_k
    where base_k is m1n = a-1 or m1p = 1-a, chosen per step so that
    |m_k| = u^k and sign(m_k) = sign(c_k) exactly (no slow 1x-mode STT ops,
    no GpSimd elementwise — it contends with DVE for SBUF ports).
  * stationary side (ScalarE): sv_k = exp(-k*lnv + ln|c_k|) fp16 — one
    activation per term, all on the single Exp table set (the table load is
    pulled to t~0 by a dummy activation and overlaps the input DMAs).
  * TensorE: K*8 accumulating matmuls (2 i-tiles x 2 o-tiles x 2 batch
    halves, N=256) into 4 PSUM banks, fp32; garbage warm-up matmuls during
    the DMA window pull the PE HAM clock gate toward 8/8 for the real work.
  * tail: y = Exp(psum + I*c0) per quadrant, emitted as bf16 (halves the
    output DMA bytes; host upcasts to fp32 — costs ~2e-3 rms, well within
    budget); bh=0 quadrants exp + DMA out while the bh=1 matmuls still run;
    all output triggers on the sync queue so they never block tail
    activations.
"""

from contextlib import ExitStack

import numpy as np

B, OUT, IN = 4096, 256, 256
NCORES = 8
B_LOC = B // NCORES  # 512 batch rows per core
K = 6
C0 = -0.00046655596782973075
CK = [
    -0.9699897586671118,
    -1.0071931168236499,
    3.1388227723833464,
    -11.164267979523085,
    15.665938740540259,
    -9.010544305490695,
]
N_WARM_MM = 6

_COMPILED = {}


def _build_nc():
    import concourse.bacc as bacc
    import concourse.mybir as mybir
    import concourse.tile as tile

    AF = mybir.ActivationFunctionType
    F32 = mybir.dt.float32
    F16 = mybir.dt.float16
    MUL = mybir.AluOpType.mult

    sgn = [1.0 if c > 0 else -1.0 for c in CK]

    nc = bacc.Bacc(
        "TRN2", target_bir_lowering=False, debug=False, num_devices=NCORES
    )

    aT = nc.dram_tensor("aT", [IN, B_LOC], F16, kind="ExternalInput").ap()
    lnvT = nc.dram_tensor("lnvT", [IN, OUT], F16, kind="ExternalInput").ap()
    BF16 = mybir.dt.bfloat16
    # partition-major output layout [p, bh, ot*256+j]: each bh-half is one
    # contiguous shape-matched DMA (2 output triggers instead of 4); the
    # host reassembles (see run()).
    y = nc.dram_tensor("y", [128, 2, B_LOC], BF16, kind="ExternalOutput").ap()

    NIT = IN // 128  # 2 i-tiles
    NOT_ = OUT // 128  # 2 o-tiles

    with tile.TileContext(nc) as tc, ExitStack() as es:
        const = es.enter_context(tc.tile_pool(name="const", bufs=1))
        mk_pool = es.enter_context(tc.tile_pool(name="mk", bufs=3))
        sv_pool = es.enter_context(tc.tile_pool(name="sv", bufs=K))
        ps_pool = es.enter_context(tc.tile_pool(name="ps", bufs=1, space="PSUM"))

        # input DMAs split across the two HWDGE rings: scalar ring carries
        # lnv it0 (triggered before the table-load dummy so the transfer
        # overlaps the load) then atoms it1; sync ring carries atoms it0 then
        # lnv it1.  ~1us trigger->packet lag, ~200GB/s striped transfers.
        lnv = const.tile([128, NIT * OUT], F16, name="lnv", tag="lnv")
        a16 = const.tile([128, NIT * B_LOC], F16, name="a16", tag="a16")
        nc.scalar.dma_start(lnv[:, 0:OUT], lnvT[0:128, :])

        # scalar queue: force the (single) Exp table load while DMAs run
        scratch = const.tile([128, 1], F32, name="scratch", tag="scratch")
        zero_ap = nc.const_aps.tensor(0.0, (128, 1))
        nc.scalar.activation(scratch[:], zero_ap, AF.Exp)

        nc.scalar.dma_start(a16[:, B_LOC : 2 * B_LOC], aT[128:256, :])
        nc.sync.dma_start(a16[:, 0:B_LOC], aT[0:128, :])
        nc.sync.dma_start(lnv[:, OUT : 2 * OUT], lnvT[128:256, :])

        # gpsimd: bias constants for the stationary activations + warm tile
        warm = const.tile([128, 512], F16, name="warm", tag="warm")
        nc.gpsimd.memset(warm[:], 0.0)
        lnck = const.tile([128, K], F32, name="lnck", tag="lnck")
        for k in range(K):
            nc.gpsimd.memset(lnck[:, k : k + 1], float(np.log(abs(CK[k]))))
        bias_c0 = const.tile([128, 1], F32, name="bias_c0", tag="bias_c0")
        nc.gpsimd.memset(bias_c0[:], float(IN * C0))

        # warm-up garbage matmuls lift the PE HAM clock gate during DMA wait
        warm_ps = ps_pool.tile([128, 512], F32, name="warm_ps", tag="warm_ps")
        for _ in range(N_WARM_MM):
            nc.tensor.matmul(
                warm_ps[:], lhsT=warm[:, 0:128], rhs=warm[:], start=True, stop=True
            )

        # stationaries: sv_k = fp16(exp(-k*lnv + ln|c_k|)), always positive;
        # the sign of c_k rides on the moving chain (see below)
        svs = []
        for k in range(1, K + 1):
            sv = sv_pool.tile([128, NIT * OUT], F16, name="sv", tag="sv")
            if k == 1:  # split halves so the first matmul starts earlier
                for it in range(NIT):
                    sl = slice(it * OUT, (it + 1) * OUT)
                    nc.scalar.activation(
                        sv[:, sl], lnv[:, sl], AF.Exp, scale=-1.0,
                        bias=lnck[:, 0:1],
                    )
            else:
                nc.scalar.activation(
                    sv[:], lnv[:], AF.Exp, scale=-float(k), bias=lnck[:, k - 1 : k]
                )
            svs.append(sv)

        # moving side: m_k = sign(c_k) * u^k via a plain-TT chain multiplying
        # by one of two base tiles: m1n = -(u) = a-1 (flips sign) or
        # m1p = +u = 1-a (keeps sign); the step-k base is chosen so that
        # sign(m_k) = sign(c_k) exactly.
        chi = [sgn[0]] + [sgn[k - 1] * sgn[k - 2] for k in range(2, K + 1)]
        need_n = any(c < 0 for c in chi)
        need_p = any(c > 0 for c in chi)
        m1n = const.tile([128, NIT * B_LOC], F16, name="m1n", tag="m1n")
        m1p = const.tile([128, NIT * B_LOC], F16, name="m1p", tag="m1p")
        primary, secondary = (m1n, m1p) if sgn[0] < 0 else (m1p, m1n)
        psc = (1.0, -1.0) if sgn[0] < 0 else (-1.0, 1.0)
        for it in range(NIT):
            sl = slice(it * B_LOC, (it + 1) * B_LOC)
            nc.vector.tensor_scalar(
                primary[:, sl], a16[:, sl], psc[0], psc[1], MUL, mybir.AluOpType.add
            )
        if need_n and need_p:
            for it in range(NIT):
                sl = slice(it * B_LOC, (it + 1) * B_LOC)
                nc.vector.tensor_scalar_mul(secondary[:, sl], primary[:, sl], -1.0)

        # one PSUM bank per (o-tile, batch-half) quadrant: the bh=0 banks
        # close during the last term, so half the output exps + DMAs overlap
        # the remaining matmuls.  Tiles are bank-sized (512 f32) with only
        # the first 256 columns used, to keep PE writes and ScalarE reads on
        # different physical banks.
        BH = B_LOC // 2  # 256
        psums = {}
        for ot in range(NOT_):
            for bh in range(2):
                psums[(ot, bh)] = ps_pool.tile(
                    [128, 512], F32, name=f"ps{ot}{bh}", tag=f"ps{ot}{bh}"
                )

        mk_prev = primary
        for k in range(1, K + 1):
            if k == 1:
                mk = primary
            else:
                base = m1n if chi[k - 1] < 0 else m1p
                mk = mk_pool.tile([128, NIT * B_LOC], F16, name="mk", tag="mk")
                nc.vector.tensor_mul(mk[:], mk_prev[:], base[:])
            mk_prev = mk
            sv = svs[k - 1]
            if k < K:
                order = [(it, ot, bh) for it in range(NIT) for ot in range(NOT_)
                         for bh in range(2)]
            else:  # last term: close the bh=0 banks first
                order = [(it, ot, bh) for bh in range(2) for it in range(NIT)
                         for ot in range(NOT_)]
            for it, ot, bh in order:
                nc.tensor.matmul(
                    psums[(ot, bh)][:, 0:BH],
                    lhsT=sv[:, it * OUT + ot * 128 : it * OUT + ot * 128 + 128],
                    rhs=mk[:, it * B_LOC + bh * BH : it * B_LOC + bh * BH + BH],
                    start=(k == 1 and it == 0),
                    stop=(k == K and it == NIT - 1),
                )

        # tail: y = exp(psum + I*c0) per quadrant into a bh-major y_sb
        # layout; one DMA per bh-half (the bh=0 half flows out while the
        # bh=1 matmuls still run).  Triggers on the sync queue only.
        y_sb = const.tile([128, NOT_ * B_LOC], BF16, name="y_sb", tag="y_sb")
        for bh in range(2):
            for ot in range(NOT_):
                sl = slice(bh * B_LOC + ot * BH, bh * B_LOC + ot * BH + BH)
                nc.scalar.activation(
                    y_sb[:, sl], psums[(ot, bh)][:, 0:BH], AF.Exp,
                    bias=bias_c0[:, 0:1],
                )
            nc.sync.dma_start(
                y[:, bh, :], y_sb[:, bh * B_LOC : (bh + 1) * B_LOC]
            )

    nc.compile()
    return nc


def get_nc():
    if "nc" not in _COMPILED:
        _COMPILED["nc"] = _build_nc()
    return _COMPILED["nc"]


def make_in_maps(atoms: np.ndarray, weights: np.ndarray):
    atoms = np.asarray(atoms)
    w32 = np.asarray(weights).astype(np.float32, copy=False)
    aT = np.ascontiguousarray(atoms.T.astype(np.float16))
    lnvT = np.ascontiguousarray(np.log1p(np.exp(-w32)).T.astype(np.float16))
    in_maps = []
    for c in range(NCORES):
        aT_sl = np.ascontiguousarray(aT[:, c * B_LOC : (c + 1) * B_LOC])
        in_maps.append({"aT": aT_sl, "lnvT": lnvT})
    return in_maps


def run(atoms: np.ndarray, weights: np.ndarray, **spmd_kwargs):
    from concourse.bass_utils import run_bass_kernel_spmd

    nc = get_nc()
    in_maps = make_in_maps(atoms, weights)
    res = run_bass_kernel_spmd(nc, in_maps, core_ids=list(range(NCORES)), **spmd_kwargs)
    out = np.empty((B, OUT), np.float32)
    for c in range(NCORES):
        yc = res.results[c]["y"].astype(np.float32)  # (128p, 2bh, 512=ot*256+j)
        yc = yc.reshape(128, 2, 2, 256)  # (p, bh, ot, j)
        # out[b, o] with b = c*512 + bh*256 + j, o = ot*128 + p
        out[c * B_LOC : (c + 1) * B_LOC, :] = (
            yc.transpose(1, 3, 2, 0).reshape(B_LOC, OUT)
        )
    return out, res


def kernel(atoms: np.ndarray, weights: np.ndarray) -> np.ndarray:
    out, _ = run(atoms, weights)
    return out


# revision 44
# speedup vs baseline: 1.0065x; 1.0065x over previous
"""Trainium2 Bass kernel for nn_LogicLayer (ProductTNorm 'and' LogicLayer forward).

Math: y[b,o] = prod_i (1 - v[o,i]*u[b,i]),  v = sigmoid(w), u = 1 - atoms.
ln y[b,o] = sum_i ln(1 - v*u) ~= I*c0 + sum_{k=1..K} c_k * sum_i v^k[o,i] u^k[b,i]
so each polynomial term is a (B,I)x(I,O) matmul and the whole reduction runs on
TensorE instead of elementwise Ln on ScalarE (the 265us baseline approach).

Coefficients c_k: weighted least-squares fit of ln(1-x) on the input
distribution (weight ~ y^2 = the norm-relative metric), fitted against the
fp16-quantized basis the device actually computes (see fit_coeffs.py).

Per-core layout (8 cores, data-parallel over batch, B_loc=512):
  * inputs: a16T = fp16(atoms.T) slice (I, B_loc), lnvT = fp16(softplus(-w).T)
    (I, O); input DMA triggers split across the sync and scalar HWDGE queues.
  * moving side (DVE): plain fp16 tensor_mul chain m_k = m_{k-1} * base_k
    where base_k is m1n = a-1 or m1p = 1-a, chosen per step so that
    |m_k| = u^k and sign(m_k) = sign(c_k) exactly (no slow 1x-mode STT ops,
    no GpSimd elementwise — it contends with DVE for SBUF ports).
  * stationary side (ScalarE): sv_k = exp(-k*lnv + ln|c_k|) fp16 — one
    activation per term, all on the single Exp table set (the table load is
    pulled to t~0 by a dummy activation and overlaps the input DMAs).
  * TensorE: K*8 accumulating matmuls (2 i-tiles x 2 o-tiles x 2 batch
    halves, N=256) into 4 PSUM banks, fp32; garbage warm-up matmuls during
    the DMA window pull the PE HAM clock gate toward 8/8 for the real work.
  * tail: y = Exp(psum + I*c0) per quadrant, emitted as bf16 (halves the
    output DMA bytes; host upcasts to fp32 — costs ~2e-3 rms, well within
    budget); bh=0 quadrants exp + DMA out while the bh=1 matmuls still run;
    all output triggers on the sync queue so they never block tail
    activations.
"""

from contextlib import ExitStack

import numpy as np

B, OUT, IN = 4096, 256, 256
NCORES = 8
B_LOC = B // NCORES  # 512 batch rows per core
K = 6
C0 = -0.00046655596782973075
CK = [
    -0.9699897586671118,
    -1.0071931168236499,
    3.1388227723833464,
    -11.164267979523085,
    15.665938740540259,
    -9.010544305490695,
]
N_WARM_MM = 7

_COMPILED = {}


def _build_nc():
    import concourse.bacc as bacc
    import concourse.mybir as mybir
    import concourse.tile as tile

    AF = mybir.ActivationFunctionType
    F32 = mybir.dt.float32
    F16 = mybir.dt.float16
    MUL = mybir.AluOpType.mult

    sgn = [1.0 if c > 0 else -1.0 for c in CK]

    nc = bacc.Bacc(
        "TRN2", target_bir_lowering=False, debug=False, num_devices=NCORES
    )

    aT = nc.dram_tensor("aT", [IN, B_LOC], F16, kind="ExternalInput").ap()
    lnvT = nc.dram_tensor("lnvT", [IN, OUT], F16, kind="ExternalInput").ap()
    BF16 = mybir.dt.bfloat16
    # partition-major output layout [p, bh, ot*256+j]: each bh-half is one
    # contiguous shape-matched DMA (2 output triggers instead of 4); the
    # host reassembles (see run()).
    y = nc.dram_tensor("y", [128, 2, B_LOC], BF16, kind="ExternalOutput").ap()

    NIT = IN // 128  # 2 i-tiles
    NOT_ = OUT // 128  # 2 o-tiles

    with tile.TileContext(nc) as tc, ExitStack() as es:
        const = es.enter_context(tc.tile_pool(name="const", bufs=1))
        mk_pool = es.enter_context(tc.tile_pool(name="mk", bufs=3))
        sv_pool = es.enter_context(tc.tile_pool(name="sv", bufs=K))
        ps_pool = es.enter_context(tc.tile_pool(name="ps", bufs=1, space="PSUM"))

        # input DMAs split across the two HWDGE rings: scalar ring carries
        # lnv it0 (triggered before the table-load dummy so the transfer
        # overlaps the load) then atoms it1; sync ring carries atoms it0 then
        # lnv it1.  ~1us trigger->packet lag, ~200GB/s striped transfers.
        lnv = const.tile([128, NIT * OUT], F16, name="lnv", tag="lnv")
        a16 = const.tile([128, NIT * B_LOC], F16, name="a16", tag="a16")
        nc.scalar.dma_start(lnv[:, 0:OUT], lnvT[0:128, :])

        # scalar queue: force the (single) Exp table load while DMAs run
        scratch = const.tile([128, 1], F32, name="scratch", tag="scratch")
        zero_ap = nc.const_aps.tensor(0.0, (128, 1))
        nc.scalar.activation(scratch[:], zero_ap, AF.Exp)

        nc.scalar.dma_start(a16[:, B_LOC : 2 * B_LOC], aT[128:256, :])
        nc.sync.dma_start(a16[:, 0:B_LOC], aT[0:128, :])
        nc.sync.dma_start(lnv[:, OUT : 2 * OUT], lnvT[128:256, :])

        # gpsimd: bias constants for the stationary activations + warm tile
        warm = const.tile([128, 512], F16, name="warm", tag="warm")
        nc.gpsimd.memset(warm[:], 0.0)
        lnck = const.tile([128, K], F32, name="lnck", tag="lnck")
        for k in range(K):
            nc.gpsimd.memset(lnck[:, k : k + 1], float(np.log(abs(CK[k]))))
        bias_c0 = const.tile([128, 1], F32, name="bias_c0", tag="bias_c0")
        nc.gpsimd.memset(bias_c0[:], float(IN * C0))

        # warm-up garbage matmuls lift the PE HAM clock gate during DMA wait
        warm_ps = ps_pool.tile([128, 512], F32, name="warm_ps", tag="warm_ps")
        for _ in range(N_WARM_MM):
            nc.tensor.matmul(
                warm_ps[:], lhsT=warm[:, 0:128], rhs=warm[:], start=True, stop=True
            )

        # stationaries: sv_k = fp16(exp(-k*lnv + ln|c_k|)), always positive;
        # the sign of c_k rides on the moving chain (see below)
        svs = []
        for k in range(1, K + 1):
            sv = sv_pool.tile([128, NIT * OUT], F16, name="sv", tag="sv")
            if k == 1:  # split halves so the first matmul starts earlier
                for it in range(NIT):
                    sl = slice(it * OUT, (it + 1) * OUT)
                    nc.scalar.activation(
                        sv[:, sl], lnv[:, sl], AF.Exp, scale=-1.0,
                        bias=lnck[:, 0:1],
                    )
            else:
                nc.scalar.activation(
                    sv[:], lnv[:], AF.Exp, scale=-float(k), bias=lnck[:, k - 1 : k]
                )
            svs.append(sv)

        # moving side: m_k = sign(c_k) * u^k via a plain-TT chain multiplying
        # by one of two base tiles: m1n = -(u) = a-1 (flips sign) or
        # m1p = +u = 1-a (keeps sign); the step-k base is chosen so that
        # sign(m_k) = sign(c_k) exactly.
        chi = [sgn[0]] + [sgn[k - 1] * sgn[k - 2] for k in range(2, K + 1)]
        need_n = any(c < 0 for c in chi)
        need_p = any(c > 0 for c in chi)
        m1n = const.tile([128, NIT * B_LOC], F16, name="m1n", tag="m1n")
        m1p = const.tile([128, NIT * B_LOC], F16, name="m1p", tag="m1p")
        primary, secondary = (m1n, m1p) if sgn[0] < 0 else (m1p, m1n)
        psc = (1.0, -1.0) if sgn[0] < 0 else (-1.0, 1.0)
        for it in range(NIT):
            sl = slice(it * B_LOC, (it + 1) * B_LOC)
            nc.vector.tensor_scalar(
                primary[:, sl], a16[:, sl], psc[0], psc[1], MUL, mybir.AluOpType.add
            )
        if need_n and need_p:
            for it in range(NIT):
                sl = slice(it * B_LOC, (it + 1) * B_LOC)
                nc.vector.tensor_scalar_mul(secondary[:, sl], primary[:, sl], -1.0)

        # one PSUM bank per (o-tile, batch-half) quadrant: the bh=0 banks
        # close during the last term, so half the output exps + DMAs overlap
        # the remaining matmuls.  Tiles are bank-sized (512 f32) with only
        # the first 256 columns used, to keep PE writes and ScalarE reads on
        # different physical banks.
        BH = B_LOC // 2  # 256
        psums = {}
        for ot in range(NOT_):
            for bh in range(2):
                psums[(ot, bh)] = ps_pool.tile(
                    [128, 512], F32, name=f"ps{ot}{bh}", tag=f"ps{ot}{bh}"
                )

        mk_prev = primary
        for k in range(1, K + 1):
            if k == 1:
                mk = primary
            else:
                base = m1n if chi[k - 1] < 0 else m1p
                mk = mk_pool.tile([128, NIT * B_LOC], F16, name="mk", tag="mk")
                nc.vector.tensor_mul(mk[:], mk_prev[:], base[:])
            mk_prev = mk
            sv = svs[k - 1]
            if k < K:
                order = [(it, ot, bh) for it in range(NIT) for ot in range(NOT_)
                         for bh in range(2)]
            else:  # last term: close the bh=0 banks first
                order = [(it, ot, bh) for bh in range(2) for it in range(NIT)
                         for ot in range(NOT_)]
            for it, ot, bh in order:
                nc.tensor.matmul(
                    psums[(ot, bh)][:, 0:BH],
                    lhsT=sv[:, it * OUT + ot * 128 : it * OUT + ot * 128 + 128],
                    rhs=mk[:, it * B_LOC + bh * BH : it * B_LOC + bh * BH + BH],
                    start=(k == 1 and it == 0),
                    stop=(k == K and it == NIT - 1),
                )

        # tail: y = exp(psum + I*c0) per quadrant into a bh-major y_sb
        # layout; one DMA per bh-half (the bh=0 half flows out while the
        # bh=1 matmuls still run).  Triggers on the sync queue only.
        y_sb = const.tile([128, NOT_ * B_LOC], BF16, name="y_sb", tag="y_sb")
        for bh in range(2):
            for ot in range(NOT_):
                sl = slice(bh * B_LOC + ot * BH, bh * B_LOC + ot * BH + BH)
                nc.scalar.activation(
                    y_sb[:, sl], psums[(ot, bh)][:, 0:BH], AF.Exp,
                    bias=bias_c0[:, 0:1],
                )
            nc.sync.dma_start(
                y[:, bh, :], y_sb[:, bh * B_LOC : (bh + 1) * B_LOC]
            )

    nc.compile()
    return nc


def get_nc():
    if "nc" not in _COMPILED:
        _COMPILED["nc"] = _build_nc()
    return _COMPILED["nc"]


def make_in_maps(atoms: np.ndarray, weights: np.ndarray):
    atoms = np.asarray(atoms)
    w32 = np.asarray(weights).astype(np.float32, copy=False)
    aT = np.ascontiguousarray(atoms.T.astype(np.float16))
    lnvT = np.ascontiguousarray(np.log1p(np.exp(-w32)).T.astype(np.float16))
    in_maps = []
    for c in range(NCORES):
        aT_sl = np.ascontiguousarray(aT[:, c * B_LOC : (c + 1) * B_LOC])
        in_maps.append({"aT": aT_sl, "lnvT": lnvT})
    return in_maps


def run(atoms: np.ndarray, weights: np.ndarray, **spmd_kwargs):
    from concourse.bass_utils import run_bass_kernel_spmd

    nc = get_nc()
    in_maps = make_in_maps(atoms, weights)
    res = run_bass_kernel_spmd(nc, in_maps, core_ids=list(range(NCORES)), **spmd_kwargs)
    out = np.empty((B, OUT), np.float32)
    for c in range(NCORES):
        yc = res.results[c]["y"].astype(np.float32)  # (128p, 2bh, 512=ot*256+j)
        yc = yc.reshape(128, 2, 2, 256)  # (p, bh, ot, j)
        # out[b, o] with b = c*512 + bh*256 + j, o = ot*128 + p
        out[c * B_LOC : (c + 1) * B_LOC, :] = (
            yc.transpose(1, 3, 2, 0).reshape(B_LOC, OUT)
        )
    return out, res


def kernel(atoms: np.ndarray, weights: np.ndarray) -> np.ndarray:
    out, _ = run(atoms, weights)
    return out


# revision 47
# speedup vs baseline: 1.0409x; 1.0342x over previous
"""Trainium2 Bass kernel for nn_LogicLayer (ProductTNorm 'and' LogicLayer forward).

Math: y[b,o] = prod_i (1 - v[o,i]*u[b,i]),  v = sigmoid(w), u = 1 - atoms.
ln y[b,o] = sum_i ln(1 - v*u) ~= I*c0 + sum_{k=1..K} c_k * sum_i v^k[o,i] u^k[b,i]
so each polynomial term is a (B,I)x(I,O) matmul and the whole reduction runs on
TensorE instead of elementwise Ln on ScalarE (the 265us baseline approach).

Coefficients c_k: weighted least-squares fit of ln(1-x) on the input
distribution (weight ~ y^2 = the norm-relative metric), fitted against the
fp16-quantized basis the device actually computes (see fit_coeffs.py).

Per-core layout (8 cores, data-parallel over batch, B_loc=512):
  * inputs: a16T = fp16(atoms.T) slice (I, B_loc), lnvT = fp16(softplus(-w).T)
    (I, O); input DMA triggers split across the sync and scalar HWDGE queues.
  * moving side (DVE): plain fp16 tensor_mul chain m_k = m_{k-1} * base_k
    where base_k is m1n = a-1 or m1p = 1-a, chosen per step so that
    |m_k| = u^k and sign(m_k) = sign(c_k) exactly (no slow 1x-mode STT ops,
    no GpSimd elementwise — it contends with DVE for SBUF ports).
  * stationary side (ScalarE): sv_k = exp(-k*lnv + ln|c_k|) fp16 — one
    activation per term, all on the single Exp table set (the table load is
    pulled to t~0 by a dummy activation and overlaps the input DMAs).
  * TensorE: K*8 accumulating matmuls (2 i-tiles x 2 o-tiles x 2 batch
    halves, N=256) into 4 PSUM banks, fp32; garbage warm-up matmuls during
    the DMA window pull the PE HAM clock gate toward 8/8 for the real work.
  * tail: y = Exp(psum + I*c0) per quadrant, emitted as bf16 (halves the
    output DMA bytes; host upcasts to fp32 — costs ~2e-3 rms, well within
    budget); bh=0 quadrants exp + DMA out while the bh=1 matmuls still run;
    all output triggers on the sync queue so they never block tail
    activations.
"""

from contextlib import ExitStack

import numpy as np

B, OUT, IN = 4096, 256, 256
NCORES = 8
B_LOC = B // NCORES  # 512 batch rows per core
K = 6
C0 = -0.00046655596782973075
CK = [
    -0.9699897586671118,
    -1.0071931168236499,
    3.1388227723833464,
    -11.164267979523085,
    15.665938740540259,
    -9.010544305490695,
]
N_WARM_MM = 7

_COMPILED = {}


def _build_nc():
    import concourse.bacc as bacc
    import concourse.mybir as mybir
    import concourse.tile as tile

    AF = mybir.ActivationFunctionType
    F32 = mybir.dt.float32
    F16 = mybir.dt.float16
    MUL = mybir.AluOpType.mult

    sgn = [1.0 if c > 0 else -1.0 for c in CK]

    nc = bacc.Bacc(
        "TRN2", target_bir_lowering=False, debug=False, num_devices=NCORES
    )

    aT = nc.dram_tensor("aT", [IN, B_LOC], F16, kind="ExternalInput").ap()
    lnvT = nc.dram_tensor("lnvT", [IN, OUT], F16, kind="ExternalInput").ap()
    BF16 = mybir.dt.bfloat16
    # partition-major output layout [p, bh, ot*256+j]: each bh-half is one
    # contiguous shape-matched DMA (2 output triggers instead of 4); the
    # host reassembles (see run()).
    y = nc.dram_tensor("y", [128, 2, B_LOC], BF16, kind="ExternalOutput").ap()

    NIT = IN // 128  # 2 i-tiles
    NOT_ = OUT // 128  # 2 o-tiles

    with tile.TileContext(nc) as tc, ExitStack() as es:
        const = es.enter_context(tc.tile_pool(name="const", bufs=1))
        mk_pool = es.enter_context(tc.tile_pool(name="mk", bufs=3))
        sv_pool = es.enter_context(tc.tile_pool(name="sv", bufs=K))
        ps_pool = es.enter_context(tc.tile_pool(name="ps", bufs=1, space="PSUM"))

        # input DMAs split across the two HWDGE rings: scalar ring carries
        # lnv it0 (triggered before the table-load dummy so the transfer
        # overlaps the load) then atoms it1; sync ring carries atoms it0 then
        # lnv it1.  ~1us trigger->packet lag, ~200GB/s striped transfers.
        lnv = const.tile([128, NIT * OUT], F16, name="lnv", tag="lnv")
        a16 = const.tile([128, NIT * B_LOC], F16, name="a16", tag="a16")
        nc.scalar.dma_start(lnv[:, 0:OUT], lnvT[0:128, :])

        # scalar queue: force the (single) Exp table load while DMAs run
        scratch = const.tile([128, 1], F32, name="scratch", tag="scratch")
        zero_ap = nc.const_aps.tensor(0.0, (128, 1))
        nc.scalar.activation(scratch[:], zero_ap, AF.Exp)

        nc.scalar.dma_start(a16[:, B_LOC : 2 * B_LOC], aT[128:256, :])
        nc.sync.dma_start(a16[:, 0:B_LOC], aT[0:128, :])
        nc.sync.dma_start(lnv[:, OUT : 2 * OUT], lnvT[128:256, :])

        # gpsimd: bias constants for the stationary activations + warm tile
        warm = const.tile([128, 512], F16, name="warm", tag="warm")
        nc.gpsimd.memset(warm[:], 0.0)
        lnck = const.tile([128, K], F32, name="lnck", tag="lnck")
        for k in range(K):
            nc.gpsimd.memset(lnck[:, k : k + 1], float(np.log(abs(CK[k]))))
        bias_c0 = const.tile([128, 1], F32, name="bias_c0", tag="bias_c0")
        nc.gpsimd.memset(bias_c0[:], float(IN * C0))

        # warm-up garbage matmuls lift the PE HAM clock gate during DMA wait
        warm_ps = ps_pool.tile([128, 512], F32, name="warm_ps", tag="warm_ps")
        for _ in range(N_WARM_MM):
            nc.tensor.matmul(
                warm_ps[:], lhsT=warm[:, 0:128], rhs=warm[:], start=True, stop=True
            )

        # stationaries: sv_k = fp16(exp(-k*lnv + ln|c_k|)), always positive;
        # the sign of c_k rides on the moving chain (see below)
        svs = []
        for k in range(1, K + 1):
            sv = sv_pool.tile([128, NIT * OUT], F16, name="sv", tag="sv")
            if k == 1:  # split halves so the first matmul starts earlier
                for it in range(NIT):
                    sl = slice(it * OUT, (it + 1) * OUT)
                    nc.scalar.activation(
                        sv[:, sl], lnv[:, sl], AF.Exp, scale=-1.0,
                        bias=lnck[:, 0:1],
                    )
            else:
                nc.scalar.activation(
                    sv[:], lnv[:], AF.Exp, scale=-float(k), bias=lnck[:, k - 1 : k]
                )
            svs.append(sv)

        # moving side: m_k = sign(c_k) * u^k via a plain-TT chain multiplying
        # by one of two base tiles: m1n = -(u) = a-1 (flips sign) or
        # m1p = +u = 1-a (keeps sign); the step-k base is chosen so that
        # sign(m_k) = sign(c_k) exactly.
        chi = [sgn[0]] + [sgn[k - 1] * sgn[k - 2] for k in range(2, K + 1)]
        need_n = any(c < 0 for c in chi)
        need_p = any(c > 0 for c in chi)
        m1n = const.tile([128, NIT * B_LOC], F16, name="m1n", tag="m1n")
        m1p = const.tile([128, NIT * B_LOC], F16, name="m1p", tag="m1p")
        primary, secondary = (m1n, m1p) if sgn[0] < 0 else (m1p, m1n)
        psc = (1.0, -1.0) if sgn[0] < 0 else (-1.0, 1.0)
        for it in range(NIT):
            sl = slice(it * B_LOC, (it + 1) * B_LOC)
            nc.vector.tensor_scalar(
                primary[:, sl], a16[:, sl], psc[0], psc[1], MUL, mybir.AluOpType.add
            )
        if need_n and need_p:
            for it in range(NIT):
                sl = slice(it * B_LOC, (it + 1) * B_LOC)
                nc.vector.tensor_scalar_mul(secondary[:, sl], primary[:, sl], -1.0)

        # one PSUM bank per batch-half, holding BOTH o-tile quadrants side
        # by side (cols ot*256+j).  The bh=0 bank closes during the last
        # term so its exp + DMA overlap the remaining matmuls, and each
        # bh-half needs only ONE tail exp (contiguous FD=512) and one sem.
        # start=True fires only on the temporally-first matmul per bank (it
        # clears has_written bank-wide); the other quadrant's first matmul
        # uses start=False and overwrites-then-sets per element.
        BH = B_LOC // 2  # 256
        psums = {}
        for bh in range(2):
            psums[bh] = ps_pool.tile(
                [128, 512], F32, name=f"ps{bh}", tag=f"ps{bh}"
            )

        mk_prev = primary
        for k in range(1, K + 1):
            if k == 1:
                mk = primary
            else:
                base = m1n if chi[k - 1] < 0 else m1p
                mk = mk_pool.tile([128, NIT * B_LOC], F16, name="mk", tag="mk")
                nc.vector.tensor_mul(mk[:], mk_prev[:], base[:])
            mk_prev = mk
            sv = svs[k - 1]
            if k < K:
                order = [(it, ot, bh) for it in range(NIT) for ot in range(NOT_)
                         for bh in range(2)]
            else:  # last term: close the bh=0 banks first
                order = [(it, ot, bh) for bh in range(2) for it in range(NIT)
                         for ot in range(NOT_)]
            for it, ot, bh in order:
                nc.tensor.matmul(
                    psums[bh][:, ot * BH : (ot + 1) * BH],
                    lhsT=sv[:, it * OUT + ot * 128 : it * OUT + ot * 128 + 128],
                    rhs=mk[:, it * B_LOC + bh * BH : it * B_LOC + bh * BH + BH],
                    start=(k == 1 and it == 0 and ot == 0),
                    stop=(k == K and it == NIT - 1 and ot == NOT_ - 1),
                )

        # tail: y = exp(psum + I*c0) per quadrant into a bh-major y_sb
        # layout; one DMA per bh-half (the bh=0 half flows out while the
        # bh=1 matmuls still run).  Triggers on the sync queue only.
        y_sb = const.tile([128, NOT_ * B_LOC], BF16, name="y_sb", tag="y_sb")
        for bh in range(2):
            nc.scalar.activation(
                y_sb[:, bh * B_LOC : (bh + 1) * B_LOC], psums[bh][:],
                AF.Exp, bias=bias_c0[:, 0:1],
            )
            nc.sync.dma_start(
                y[:, bh, :], y_sb[:, bh * B_LOC : (bh + 1) * B_LOC]
            )

    nc.compile()
    return nc


def get_nc():
    if "nc" not in _COMPILED:
        _COMPILED["nc"] = _build_nc()
    return _COMPILED["nc"]


def make_in_maps(atoms: np.ndarray, weights: np.ndarray):
    atoms = np.asarray(atoms)
    w32 = np.asarray(weights).astype(np.float32, copy=False)
    aT = np.ascontiguousarray(atoms.T.astype(np.float16))
    lnvT = np.ascontiguousarray(np.log1p(np.exp(-w32)).T.astype(np.float16))
    in_maps = []
    for c in range(NCORES):
        aT_sl = np.ascontiguousarray(aT[:, c * B_LOC : (c + 1) * B_LOC])
        in_maps.append({"aT": aT_sl, "lnvT": lnvT})
    return in_maps


def run(atoms: np.ndarray, weights: np.ndarray, **spmd_kwargs):
    from concourse.bass_utils import run_bass_kernel_spmd

    nc = get_nc()
    in_maps = make_in_maps(atoms, weights)
    res = run_bass_kernel_spmd(nc, in_maps, core_ids=list(range(NCORES)), **spmd_kwargs)
    out = np.empty((B, OUT), np.float32)
    for c in range(NCORES):
        yc = res.results[c]["y"].astype(np.float32)  # (128p, 2bh, 512=ot*256+j)
        yc = yc.reshape(128, 2, 2, 256)  # (p, bh, ot, j)
        # out[b, o] with b = c*512 + bh*256 + j, o = ot*128 + p
        out[c * B_LOC : (c + 1) * B_LOC, :] = (
            yc.transpose(1, 3, 2, 0).reshape(B_LOC, OUT)
        )
    return out, res


def kernel(atoms: np.ndarray, weights: np.ndarray) -> np.ndarray:
    out, _ = run(atoms, weights)
    return out
